# revision 57
# baseline (speedup 1.0000x reference)
"""Trainium2 Bass kernel for nn_Attention_50457275793495.

Per-head attention with bias injected into logits, sigmoid gating, and an
output projection.  Returns BOTH the projected output (B, L, O) and the full
normalized attention tensor (B, Lq, Lk, H).

Sharding (8 cores): core c handles batch b = c // 4 and the two heads
(2*(c%4), 2*(c%4)+1).  Each core:
  - projects q/kv inputs to Q^T, G^T, K^T, V (per head),
  - computes S = (Q * C^-1/2) K^T + bias via PE (bias accumulated into PSUM
    with an identity-matmul), exp on ScalarE with fused row-sum accumulation,
    normalizes on VectorE, streams the attention matrix back to HBM,
  - transposes attention blocks on PE and computes out^T = V^T A^T,
  - gates with sigmoid(G^T) and accumulates the per-head output projection.
Host sums the 8 partial projection outputs (pure unshard reduction) and
interleaves per-head attention slabs into the (B, Lq, Lk, H) output.

Shapes hardcoded: B=2, L=2048, D=256, H=8, C=32, A=1, O=256.
"""

import numpy as np

B, L, D, H, C, O = 2, 2048, 256, 8, 32, 256
C2 = 2 * C
P = 128
NT = L // P            # 16 q-tiles of 128 rows
NCH = NT // 4          # 4 chunks of 4 q-tiles
NCORES = 8
RS = float(C) ** -0.5  # 1/sqrt(32)

# dtype config for PE matmul sites: float32r runs 4x faster on TRN2's PE
# than float32 (single-pass vs 2x half-speed passes).
F32R_S = True       # S = Q K^T
F32R_BIAS = True    # identity-matmul bias accumulate
F32R_AV = True      # out^T = V^T A^T
F32R_TRANS = True   # E tiles + PE transposes of the attention matrix
F32R_PROJ = True    # qg/kv/output projections

_CACHE = {}


def _build_program():
    import concourse.bass as bass
    import concourse.tile as tile
    from concourse import bacc, mybir
    from concourse.masks import make_identity
    from contextlib import ExitStack

    f32 = mybir.dt.float32
    f32r = mybir.dt.float32r
    AF = mybir.ActivationFunctionType
    ALU = mybir.AluOpType

    # Bacc (not plain Bass): its compile() pass legalizes instructions that
    # carry more sync-waits than the HW instruction structs support.
    nc = bacc.Bacc("TRN2", target_bir_lowering=False, debug=False)

    # dtype used for fast-matmul operands: producers must write f32r-rounded
    # values (BIR verifier enforces this), so the tiles themselves carry the
    # float32r dtype and the producing DVE ops round on write.
    d_s = f32r if F32R_S else f32
    d_bias = f32r if F32R_BIAS else f32
    d_av = f32r if F32R_AV else f32
    d_e = f32r if F32R_TRANS else f32
    d_p = f32r if F32R_PROJ else f32

    xq_d = nc.dram_tensor("xq", [L, D], f32, kind="ExternalInput").ap()
    xkv_d = nc.dram_tensor("xkv", [L, D], f32, kind="ExternalInput").ap()
    bias_d = nc.dram_tensor("biasH", [2, L, L], d_bias, kind="ExternalInput").ap()
    wqg_d = nc.dram_tensor("wqg", [2, D, C2], d_p, kind="ExternalInput").ap()
    wkv_d = nc.dram_tensor("wkv", [2, D, C2], d_p, kind="ExternalInput").ap()
    bqg_d = nc.dram_tensor("bqg", [2, C2], f32, kind="ExternalInput").ap()
    bkv_d = nc.dram_tensor("bkv", [2, C2], f32, kind="ExternalInput").ap()
    wo_d = nc.dram_tensor("wo", [2, C, O], d_p, kind="ExternalInput").ap()
    ob_d = nc.dram_tensor("ob", [1, O], f32, kind="ExternalInput").ap()
    attn_d = nc.dram_tensor("attn_o", [2, L, L], d_e, kind="ExternalOutput").ap()
    y_d = nc.dram_tensor("y_o", [L, O], f32, kind="ExternalOutput").ap()

    def rr(ap, on):
        return ap.bitcast(f32r) if on else ap

    with ExitStack() as ctx:
        tc = ctx.enter_context(tile.TileContext(nc))
        # SBUF pools
        const = ctx.enter_context(tc.tile_pool(name="const", bufs=1))
        xt_pool = ctx.enter_context(tc.tile_pool(name="xt", bufs=1))
        # NOTE: pools whose tiles are written by HWDGE DMAs must have a
        # slot-reuse distance that is a multiple of 8 DMA issues — the DMA
        # instruction supports only 2 sync-waits, and reuse at a multiple of
        # 8 merges the WAW wait with the own-queue ring wait.
        xload = ctx.enter_context(tc.tile_pool(name="xload", bufs=4))
        pr_pool = ctx.enter_context(tc.tile_pool(name="pr", bufs=1))
        og_pool = ctx.enter_context(tc.tile_pool(name="og", bufs=2))
        bias_pool = ctx.enter_context(tc.tile_pool(name="biasp", bufs=4))
        e_pool = ctx.enter_context(tc.tile_pool(name="ep", bufs=5))
        at_pool = ctx.enter_context(tc.tile_pool(name="atp", bufs=4))
        small = ctx.enter_context(tc.tile_pool(name="small", bufs=4))
        small2 = ctx.enter_context(tc.tile_pool(name="small2", bufs=2))
        ypool = ctx.enter_context(tc.tile_pool(name="yp", bufs=2))
        # PSUM pools: 2x2 + 2 + 2 = 8 banks
        s_pool = ctx.enter_context(tc.tile_pool(name="spsum", bufs=2, space="PSUM"))
        tp_pool = ctx.enter_context(tc.tile_pool(name="tpsum", bufs=2, space="PSUM"))
        ot_pool = ctx.enter_context(tc.tile_pool(name="otpsum", bufs=2, space="PSUM"))

        # --- constants / weights ---
        ident = const.tile([P, P], f32)
        make_identity(nc, ident)
        ident_r = const.tile([P, P], d_bias)
        nc.vector.tensor_copy(ident_r, ident)
        # PE observation op: the HW matmul's LW slot supports a single
        # sync-wait, so make PE observe the gpsimd tick (identity) up front;
        # later PE instructions then only ever wait on one new semaphore.
        warm = tp_pool.tile([P, 512], f32, tag="tp")
        nc.tensor.transpose(warm[0:1, 0:P], ident[:, 0:1], ident)
        ones32 = const.tile([1, C], f32)
        nc.vector.memset(ones32, 1.0)

        # --- phase 0: transpose inputs to x^T [d, l] (2 d-tiles each) ---
        # The x loads are issued BEFORE the constant loads so PE's first
        # transposes aren't stuck behind slow broadcast DMAs at startup.
        xqT = xt_pool.tile([P, 2, L], d_p)
        xkT = xt_pool.tile([P, 2, L], d_p)
        for src_d, dst in ((xq_d, xqT), (xkv_d, xkT)):
            for g in range(4):
                tp0 = tp_pool.tile([P, 512], f32, tag="tp")
                tp1 = tp_pool.tile([P, 512], f32, tag="tp")
                for t4 in range(4):
                    t = 4 * g + t4
                    xt_t = xload.tile([P, D], f32, tag="xl")
                    nc.sync.dma_start(out=xt_t, in_=src_d[t * P:(t + 1) * P, :])
                    nc.tensor.transpose(tp0[:, t4 * P:(t4 + 1) * P], xt_t[:, 0:P], ident)
                    nc.tensor.transpose(tp1[:, t4 * P:(t4 + 1) * P], xt_t[:, P:D], ident)
                nc.any.tensor_copy(dst[:, 0, g * 512:(g + 1) * 512], tp0)
                nc.any.tensor_copy(dst[:, 1, g * 512:(g + 1) * 512], tp1)

        b256 = const.tile([P, O], f32)
        nc.sync.dma_start(out=b256, in_=ob_d.to_broadcast([P, O]))
        wq_sb = const.tile([P, 2, 2, C2], d_p)   # [d-part, d-tile j, head hh, c2]
        wk_sb = const.tile([P, 2, 2, C2], d_p)
        for j in range(2):
            for hh in range(2):
                nc.sync.dma_start(out=wq_sb[:, j, hh, :], in_=wqg_d[hh, j * P:(j + 1) * P, :])
                nc.sync.dma_start(out=wk_sb[:, j, hh, :], in_=wkv_d[hh, j * P:(j + 1) * P, :])
        bq_sb = const.tile([C2, 2], f32)
        bk_sb = const.tile([C2, 2], f32)
        for hh in range(2):
            nc.sync.dma_start(out=bq_sb[:, hh:hh + 1], in_=bqg_d[hh:hh + 1, :].rearrange("a c -> c a"))
            nc.sync.dma_start(out=bk_sb[:, hh:hh + 1], in_=bkv_d[hh:hh + 1, :].rearrange("a c -> c a"))
        wo_sb = const.tile([C, 2, O], d_p)
        for hh in range(2):
            nc.sync.dma_start(out=wo_sb[:, hh, :], in_=wo_d[hh])
        # PE observation op: see above — have PE observe the DVE tick of the
        # x^T copies so the first projection matmul only waits on its weights
        # DMA.
        warm2 = tp_pool.tile([P, 512], d_p, tag="tp")
        nc.tensor.transpose(warm2[0:1, 0:P], xkT[:, 1, L - 1:L],
                            ident_r if F32R_PROJ else ident)

        og_tiles = []
        for hh in range(2):
            # --- phase A: projections for this pair ---
            qt = pr_pool.tile([C, L], d_s, tag="qt")
            gt = pr_pool.tile([C, L], f32, tag="gt")
            kt = pr_pool.tile([C, L], d_s, tag="kt")
            vt = pr_pool.tile([C, L], d_av, tag="vt")
            for n in range(4):
                ns = slice(n * 512, (n + 1) * 512)
                pq = tp_pool.tile([P, 512], f32, tag="tp")
                for j in range(2):
                    nc.tensor.matmul(pq[0:C2, :], lhsT=wq_sb[:, j, hh, :],
                                     rhs=xqT[:, j, ns],
                                     start=(j == 0), stop=(j == 1))
                nc.vector.tensor_scalar(qt[:, ns], pq[0:C, :], bq_sb[0:C, hh:hh + 1], RS,
                                        op0=ALU.add, op1=ALU.mult)
                nc.vector.tensor_scalar_add(gt[:, ns], pq[C:C2, :], bq_sb[C:C2, hh:hh + 1])
            for n in range(4):
                ns = slice(n * 512, (n + 1) * 512)
                pk = tp_pool.tile([P, 512], f32, tag="tp")
                for j in range(2):
                    nc.tensor.matmul(pk[0:C2, :], lhsT=wk_sb[:, j, hh, :],
                                     rhs=xkT[:, j, ns],
                                     start=(j == 0), stop=(j == 1))
                nc.vector.tensor_scalar_add(kt[:, ns], pk[0:C, :], bk_sb[0:C, hh:hh + 1])
                nc.vector.tensor_scalar_add(vt[:, ns], pk[C:C2, :], bk_sb[C:C2, hh:hh + 1])
            sig = pr_pool.tile([C, L], f32, tag="sig")
            nc.scalar.activation(sig, gt, AF.Sigmoid)
            # V^T [c, l] -> V [l, c] blocks [128, 32]
            vsb = pr_pool.tile([P, NT, C], d_av, tag="vsb")
            for g in range(4):
                tpv = tp_pool.tile([P, 512], d_av, tag="tp")
                for t4 in range(4):
                    j = 4 * g + t4
                    nc.tensor.transpose(tpv[:, t4 * C:(t4 + 1) * C],
                                        vt[:, j * P:(j + 1) * P],
                                        (ident_r if F32R_AV else ident)[0:C, 0:C])
                nc.vector.tensor_copy(vsb[:, 4 * g:4 * (g + 1), :], tpv[:, 0:4 * C])

            ogt = og_pool.tile([C, L], d_p, tag="og")
            og_tiles.append(ogt)

            # --- phase B: attention, software-pipelined at 2-q-tile chunks ---
            # Per chunk (2 q-tiles, 256 q): while the current chunk's S
            # matmuls/exps run, the PREVIOUS chunk's 16 transpose groups and
            # AV matmuls are interleaved between them so PE never idles on
            # the exp latency. AV consumes UNNORMALIZED E^T; out^T gets a
            # 1/rowsum broadcast fixup per chunk; attention tiles are
            # normalized in place afterwards, gating only their DMA store.
            NCH2 = NT // 2
            QW = 256  # chunk width in q

            def emit_y_tile(t):
                tsl = slice(t * P, (t + 1) * P)
                yp = tp_pool.tile([P, 512], f32, tag="tp")
                nc.tensor.matmul(yp[:, 0:O], lhsT=og_tiles[0][:, tsl],
                                 rhs=wo_sb[:, 0, :], start=True, stop=False)
                nc.tensor.matmul(yp[:, 0:O], lhsT=og_tiles[1][:, tsl],
                                 rhs=wo_sb[:, 1, :], start=False, stop=True)
                ysb = ypool.tile([P, O], f32, tag="ysb")
                nc.vector.tensor_add(ysb, yp[:, 0:O], b256)
                nc.sync.dma_start(out=y_d[tsl, :], in_=ysb)

            class Chunk:
                pass

            def emit_s_tile(cch, t2):
                t = 2 * cch + t2
                tsl = slice(t * P, (t + 1) * P)
                bias_sb = bias_pool.tile([P, L], d_bias, tag="bias")
                nc.sync.dma_start(out=bias_sb, in_=bias_d[hh, tsl, :])
                E = e_pool.tile([P, L], d_e, tag="e")
                acc = small.tile([P, 2], f32, tag="acc")
                qslice = qt[:, tsl]
                # S matmuls for both halves first (one Q^T load), then both
                # halves' bias matmuls (one identity load) — the repeated
                # fused LDWEIGHTS is the expensive part.
                S_half = []
                for half in range(2):
                    S = s_pool.tile([P, 1024], f32, tag="s")
                    S_half.append(S)
                    for n in range(2):
                        ns = slice(half * 1024 + n * 512,
                                   half * 1024 + (n + 1) * 512)
                        nc.tensor.matmul(S[:, n * 512:(n + 1) * 512],
                                         lhsT=qslice, rhs=kt[:, ns],
                                         start=True, stop=False)
                for half in range(2):
                    for n in range(2):
                        ns = slice(half * 1024 + n * 512,
                                   half * 1024 + (n + 1) * 512)
                        nc.tensor.matmul(S_half[half][:, n * 512:(n + 1) * 512],
                                         lhsT=ident_r, rhs=bias_sb[:, ns],
                                         start=False, stop=True)
                    hs = slice(half * 1024, (half + 1) * 1024)
                    nc.scalar.activation(E[:, hs], S_half[half], AF.Exp,
                                         accum_out=acc[:, half:half + 1])
                rsum = small.tile([P, 1], f32, tag="rsum")
                nc.vector.tensor_add(rsum, acc[:, 0:1], acc[:, 1:2])
                rec = small.tile([P, 1], f32, tag="rec")
                nc.vector.reciprocal(rec, rsum)
                return E, rec

            def start_chunk(cch):
                ck = Chunk()
                ck.cch = cch
                ck.e = []
                ck.rec = []
                ck.ot = ot_pool.tile([C, QW], f32, tag="ot")
                ck.at = []
                ck.next_av = 0
                return ck

            def emit_group(ck, j):
                # one k-block: 2 transposes -> copy -> (AV two groups back)
                tpa = tp_pool.tile([P, QW], d_e, tag="tp")
                for t2 in range(2):
                    nc.tensor.transpose(tpa[:, t2 * P:(t2 + 1) * P],
                                        ck.e[t2][:, j * P:(j + 1) * P],
                                        ident_r if F32R_TRANS else ident)
                aT = at_pool.tile([P, QW], d_av, tag="at")
                nc.vector.tensor_copy(aT, tpa)
                ck.at.append(aT)
                while ck.next_av <= len(ck.at) - 3:
                    jj = ck.next_av
                    nc.tensor.matmul(ck.ot, lhsT=vsb[:, jj, :], rhs=ck.at[jj],
                                     start=(jj == 0), stop=False)
                    ck.next_av += 1

            def emit_rts(ck):
                # 1/rowsum transposed to [1, QW] — emitted early so the
                # ones-matmul broadcast in finish_chunk never stalls PE.
                rtp = tp_pool.tile([P, QW], f32, tag="tp")
                for t2 in range(2):
                    nc.tensor.transpose(rtp[0:1, t2 * P:(t2 + 1) * P],
                                        ck.rec[t2], ident)
                ck.rts = small2.tile([1, QW], f32, tag="rts")
                nc.vector.tensor_copy(ck.rts, rtp[0:1, :])

            def finish_chunk(ck):
                for jj in range(ck.next_av, 16):
                    nc.tensor.matmul(ck.ot, lhsT=vsb[:, jj, :], rhs=ck.at[jj],
                                     start=(jj == 0), stop=(jj == 15))
                # R32[c, q-chunk] = broadcast of 1/rowsum over 32 partitions
                r32p = ot_pool.tile([C, QW], f32, tag="ot")
                nc.tensor.matmul(r32p, lhsT=ones32, rhs=ck.rts,
                                 start=True, stop=True)
                r32 = small2.tile([C, QW], f32, tag="r32")
                nc.vector.tensor_copy(r32, r32p)
                csl = slice(ck.cch * QW, (ck.cch + 1) * QW)
                nc.vector.tensor_mul(r32, ck.ot, r32)
                nc.vector.tensor_mul(ogt[:, csl], r32, sig[:, csl])
                for t2 in range(2):
                    t = 2 * ck.cch + t2
                    nc.vector.tensor_scalar_mul(ck.e[t2], ck.e[t2], ck.rec[t2])
                    nc.sync.dma_start(out=attn_d[hh, t * P:(t + 1) * P, :],
                                      in_=ck.e[t2])

            prev = None
            for cch in range(NCH2):
                cur = start_chunk(cch)
                for t2 in range(2):
                    E, rec = emit_s_tile(cch, t2)
                    cur.e.append(E)
                    cur.rec.append(rec)
                    if prev is not None:
                        if t2 == 1:
                            emit_rts(prev)
                        for j in range(8 * t2, 8 * t2 + 8):
                            emit_group(prev, j)
                if prev is not None:
                    finish_chunk(prev)
                    if hh == 1 and prev.cch >= 1:
                        emit_y_tile(2 * (prev.cch - 1))
                        emit_y_tile(2 * (prev.cch - 1) + 1)
                prev = cur
            # epilogue: drain the in-flight chunk + remaining y tiles
            for j in range(16):
                if j == 14:
                    emit_rts(prev)
                emit_group(prev, j)
            finish_chunk(prev)
            if hh == 1:
                for t in range(2 * (NCH2 - 2), NT):
                    emit_y_tile(t)

    nc.compile()
    return nc


def _get_nc():
    if "nc" not in _CACHE:
        _CACHE["nc"] = _build_program()
    return _CACHE["nc"]


def make_in_maps(q_inputs, kv_inputs, bias, qg_weights, kv_weights,
                 qg_bias, kv_bias, o_weights, o_bias):
    f = np.float32
    q_inputs = np.asarray(q_inputs, f)
    kv_inputs = np.asarray(kv_inputs, f)
    bias = np.asarray(bias, f)
    qg_weights = np.asarray(qg_weights, f)
    kv_weights = np.asarray(kv_weights, f)
    qg_bias = np.asarray(qg_bias, f)
    kv_bias = np.asarray(kv_bias, f)
    o_weights = np.asarray(o_weights, f)
    o_bias = np.asarray(o_bias, f)
    in_maps = []
    for c in range(NCORES):
        b, i = c // 4, c % 4
        hs = slice(2 * i, 2 * i + 2)
        in_maps.append({
            "xq": q_inputs[b],
            "xkv": kv_inputs[b],
            "biasH": bias[b, hs],
            "wqg": np.ascontiguousarray(np.moveaxis(qg_weights[:, 0, hs, :], 1, 0)),
            "wkv": np.ascontiguousarray(np.moveaxis(kv_weights[:, 0, hs, :], 1, 0)),
            "bqg": np.ascontiguousarray(qg_bias[0, hs, 0, :]),
            "bkv": np.ascontiguousarray(kv_bias[0, hs, 0, :]),
            "wo": np.ascontiguousarray(o_weights[0, hs]),
            "ob": (np.ascontiguousarray(o_bias[:, 0:1].T) if c in (0, 4)
                   else np.zeros((1, O), f)),
        })
    return in_maps


def assemble(results):
    y = np.zeros((B, L, O), np.float32)
    attn = np.empty((B, L, L, H), np.float32)
    for c in range(NCORES):
        b, i = c // 4, c % 4
        y[b] += results[c]["y_o"]
        a = results[c]["attn_o"]
        attn[b, :, :, 2 * i] = a[0]
        attn[b, :, :, 2 * i + 1] = a[1]
    return y, attn


def kernel(**inputs):
    from concourse.bass_utils import run_bass_kernel_spmd
    nc = _get_nc()
    in_maps = make_in_maps(**inputs)
    res = run_bass_kernel_spmd(nc, in_maps, list(range(NCORES)))
    return assemble(res.results)


def kernel_profiled(**inputs):
    """Like kernel(), but captures an NTFF profile. Returns (outputs, exec_ns, trace)."""
    from concourse.bass_utils import run_bass_kernel_spmd
    nc = _get_nc()
    in_maps = make_in_maps(**inputs)
    res = run_bass_kernel_spmd(nc, in_maps, list(range(NCORES)), trace=True)
    trace_path = None
    if res.instructions_and_trace is not None:
        trace_path = res.instructions_and_trace[1]
    return assemble(res.results), res.exec_time_ns, trace_path


# revision 60
# speedup vs baseline: 1.1901x; 1.1901x over previous
"""Trainium2 Bass kernel for nn_Attention_50457275793495.

Per-head attention with bias injected into logits, sigmoid gating, and an
output projection.  Returns BOTH the projected output (B, L, O) and the full
normalized attention tensor (B, Lq, Lk, H).

Sharding (8 cores): core c handles batch b = c // 4 and the two heads
(2*(c%4), 2*(c%4)+1).  Each core:
  - projects q/kv inputs to Q^T, G^T, K^T, V (per head),
  - computes S = (Q * C^-1/2) K^T + bias via PE (bias accumulated into PSUM
    with an identity-matmul), exp on ScalarE with fused row-sum accumulation,
    normalizes on VectorE, streams the attention matrix back to HBM,
  - transposes attention blocks on PE and computes out^T = V^T A^T,
  - gates with sigmoid(G^T) and accumulates the per-head output projection.
Host sums the 8 partial projection outputs (pure unshard reduction) and
interleaves per-head attention slabs into the (B, Lq, Lk, H) output.

Shapes hardcoded: B=2, L=2048, D=256, H=8, C=32, A=1, O=256.
"""

import numpy as np

B, L, D, H, C, O = 2, 2048, 256, 8, 32, 256
C2 = 2 * C
P = 128
NT = L // P            # 16 q-tiles of 128 rows
NCH = NT // 4          # 4 chunks of 4 q-tiles
NCORES = 8
RS = float(C) ** -0.5  # 1/sqrt(32)

# dtype config for PE matmul sites: float32r runs 4x faster on TRN2's PE
# than float32 (single-pass vs 2x half-speed passes).
F32R_S = True       # S = Q K^T
F32R_BIAS = True    # identity-matmul bias accumulate
F32R_AV = True      # out^T = V^T A^T
F32R_TRANS = True   # E tiles + PE transposes of the attention matrix
F32R_PROJ = True    # qg/kv/output projections

_CACHE = {}


def _build_program():
    import concourse.bass as bass
    import concourse.tile as tile
    from concourse import bacc, mybir
    from concourse.masks import make_identity
    from contextlib import ExitStack

    f32 = mybir.dt.float32
    f32r = mybir.dt.float32r
    AF = mybir.ActivationFunctionType
    ALU = mybir.AluOpType

    # Bacc (not plain Bass): its compile() pass legalizes instructions that
    # carry more sync-waits than the HW instruction structs support.
    nc = bacc.Bacc("TRN2", target_bir_lowering=False, debug=False)

    # dtype used for fast-matmul operands: producers must write f32r-rounded
    # values (BIR verifier enforces this), so the tiles themselves carry the
    # float32r dtype and the producing DVE ops round on write.
    d_s = f32r if F32R_S else f32
    d_bias = f32r if F32R_BIAS else f32
    d_av = f32r if F32R_AV else f32
    d_e = f32r if F32R_TRANS else f32
    d_p = f32r if F32R_PROJ else f32

    xq_d = nc.dram_tensor("xq", [L, D], f32, kind="ExternalInput").ap()
    xkv_d = nc.dram_tensor("xkv", [L, D], f32, kind="ExternalInput").ap()
    bias_d = nc.dram_tensor("biasH", [2, L, L], d_bias, kind="ExternalInput").ap()
    wqg_d = nc.dram_tensor("wqg", [2, D, C2], d_p, kind="ExternalInput").ap()
    wkv_d = nc.dram_tensor("wkv", [2, D, C2], d_p, kind="ExternalInput").ap()
    bqg_d = nc.dram_tensor("bqg", [2, C2], f32, kind="ExternalInput").ap()
    bkv_d = nc.dram_tensor("bkv", [2, C2], f32, kind="ExternalInput").ap()
    wo_d = nc.dram_tensor("wo", [2, C, O], d_p, kind="ExternalInput").ap()
    ob_d = nc.dram_tensor("ob", [1, O], f32, kind="ExternalInput").ap()
    attn_d = nc.dram_tensor("attn_o", [2, L, L], d_e, kind="ExternalOutput").ap()
    y_d = nc.dram_tensor("y_o", [L, O], f32, kind="ExternalOutput").ap()

    def rr(ap, on):
        return ap.bitcast(f32r) if on else ap

    with ExitStack() as ctx:
        tc = ctx.enter_context(tile.TileContext(nc))
        # SBUF pools
        const = ctx.enter_context(tc.tile_pool(name="const", bufs=1))
        xt_pool = ctx.enter_context(tc.tile_pool(name="xt", bufs=1))
        # NOTE: pools whose tiles are written by HWDGE DMAs must have a
        # slot-reuse distance that is a multiple of 8 DMA issues — the DMA
        # instruction supports only 2 sync-waits, and reuse at a multiple of
        # 8 merges the WAW wait with the own-queue ring wait.
        xload = ctx.enter_context(tc.tile_pool(name="xload", bufs=4))
        pr_pool = ctx.enter_context(tc.tile_pool(name="pr", bufs=1))
        og_pool = ctx.enter_context(tc.tile_pool(name="og", bufs=2))
        bias_pool = ctx.enter_context(tc.tile_pool(name="biasp", bufs=4))
        e_pool = ctx.enter_context(tc.tile_pool(name="ep", bufs=5))
        at_pool = ctx.enter_context(tc.tile_pool(name="atp", bufs=4))
        small = ctx.enter_context(tc.tile_pool(name="small", bufs=4))
        small2 = ctx.enter_context(tc.tile_pool(name="small2", bufs=2))
        ypool = ctx.enter_context(tc.tile_pool(name="yp", bufs=2))
        # PSUM pools: 2x2 + 2 + 2 = 8 banks
        s_pool = ctx.enter_context(tc.tile_pool(name="spsum", bufs=2, space="PSUM"))
        tp_pool = ctx.enter_context(tc.tile_pool(name="tpsum", bufs=2, space="PSUM"))
        ot_pool = ctx.enter_context(tc.tile_pool(name="otpsum", bufs=2, space="PSUM"))

        # --- constants / weights ---
        ident = const.tile([P, P], f32)
        make_identity(nc, ident)
        ident_r = const.tile([P, P], d_bias)
        nc.vector.tensor_copy(ident_r, ident)
        # PE observation op: the HW matmul's LW slot supports a single
        # sync-wait, so make PE observe the gpsimd tick (identity) up front;
        # later PE instructions then only ever wait on one new semaphore.
        warm = tp_pool.tile([P, 512], f32, tag="tp")
        nc.tensor.transpose(warm[0:1, 0:P], ident[:, 0:1], ident)
        ones32 = const.tile([1, C], f32)
        nc.vector.memset(ones32, 1.0)

        # --- phase 0: transpose inputs to x^T [d, l] (2 d-tiles each) ---
        # The x loads are issued BEFORE the constant loads so PE's first
        # transposes aren't stuck behind slow broadcast DMAs at startup.
        xqT = xt_pool.tile([P, 2, L], d_p)
        xkT = xt_pool.tile([P, 2, L], d_p)
        for src_d, dst in ((xq_d, xqT), (xkv_d, xkT)):
            for g in range(4):
                tp0 = tp_pool.tile([P, 512], f32, tag="tp")
                tp1 = tp_pool.tile([P, 512], f32, tag="tp")
                for t4 in range(4):
                    t = 4 * g + t4
                    xt_t = xload.tile([P, D], f32, tag="xl")
                    nc.sync.dma_start(out=xt_t, in_=src_d[t * P:(t + 1) * P, :])
                    nc.tensor.transpose(tp0[:, t4 * P:(t4 + 1) * P], xt_t[:, 0:P], ident)
                    nc.tensor.transpose(tp1[:, t4 * P:(t4 + 1) * P], xt_t[:, P:D], ident)
                nc.any.tensor_copy(dst[:, 0, g * 512:(g + 1) * 512], tp0)
                nc.any.tensor_copy(dst[:, 1, g * 512:(g + 1) * 512], tp1)

        b256 = const.tile([P, O], f32)
        nc.sync.dma_start(out=b256, in_=ob_d.to_broadcast([P, O]))
        wq_sb = const.tile([P, 2, 2, C2], d_p)   # [d-part, d-tile j, head hh, c2]
        wk_sb = const.tile([P, 2, 2, C2], d_p)
        for j in range(2):
            for hh in range(2):
                nc.sync.dma_start(out=wq_sb[:, j, hh, :], in_=wqg_d[hh, j * P:(j + 1) * P, :])
                nc.sync.dma_start(out=wk_sb[:, j, hh, :], in_=wkv_d[hh, j * P:(j + 1) * P, :])
        bq_sb = const.tile([C2, 2], f32)
        bk_sb = const.tile([C2, 2], f32)
        for hh in range(2):
            nc.sync.dma_start(out=bq_sb[:, hh:hh + 1], in_=bqg_d[hh:hh + 1, :].rearrange("a c -> c a"))
            nc.sync.dma_start(out=bk_sb[:, hh:hh + 1], in_=bkv_d[hh:hh + 1, :].rearrange("a c -> c a"))
        wo_sb = const.tile([C, 2, O], d_p)
        for hh in range(2):
            nc.sync.dma_start(out=wo_sb[:, hh, :], in_=wo_d[hh])
        # PE observation op: see above — have PE observe the DVE tick of the
        # x^T copies so the first projection matmul only waits on its weights
        # DMA.
        warm2 = tp_pool.tile([P, 512], d_p, tag="tp")
        nc.tensor.transpose(warm2[0:1, 0:P], xkT[:, 1, L - 1:L],
                            ident_r if F32R_PROJ else ident)

        og_tiles = []
        for hh in range(2):
            # --- phase A: projections for this pair ---
            qt = pr_pool.tile([C, L], d_s, tag="qt")
            gt = pr_pool.tile([C, L], f32, tag="gt")
            kt = pr_pool.tile([C, L], d_s, tag="kt")
            vt = pr_pool.tile([C, L], d_av, tag="vt")
            # q/g evacuations run on the (idle) ScalarE so phase A isn't
            # DVE-paced; bias must be pre-scaled since ACT computes
            # func(in*scale + bias).
            bqRS = small.tile([C, 1], f32, tag="rec")
            nc.vector.tensor_scalar_mul(bqRS, bq_sb[0:C, hh:hh + 1], RS)
            for n in range(4):
                ns = slice(n * 512, (n + 1) * 512)
                pq = tp_pool.tile([P, 512], f32, tag="tp")
                for j in range(2):
                    nc.tensor.matmul(pq[0:C2, :], lhsT=wq_sb[:, j, hh, :],
                                     rhs=xqT[:, j, ns],
                                     start=(j == 0), stop=(j == 1))
                nc.scalar.activation(qt[:, ns], pq[0:C, :], AF.Identity,
                                     bias=bqRS, scale=RS)
                nc.scalar.activation(gt[:, ns], pq[C:C2, :], AF.Identity,
                                     bias=bq_sb[C:C2, hh:hh + 1])
            for n in range(4):
                ns = slice(n * 512, (n + 1) * 512)
                pk = tp_pool.tile([P, 512], f32, tag="tp")
                for j in range(2):
                    nc.tensor.matmul(pk[0:C2, :], lhsT=wk_sb[:, j, hh, :],
                                     rhs=xkT[:, j, ns],
                                     start=(j == 0), stop=(j == 1))
                nc.vector.tensor_scalar_add(kt[:, ns], pk[0:C, :], bk_sb[0:C, hh:hh + 1])
                nc.vector.tensor_scalar_add(vt[:, ns], pk[C:C2, :], bk_sb[C:C2, hh:hh + 1])
            sig = pr_pool.tile([C, L], f32, tag="sig")
            nc.scalar.activation(sig, gt, AF.Sigmoid)
            # V^T [c, l] -> V [l, c] blocks [128, 32]
            vsb = pr_pool.tile([P, NT, C], d_av, tag="vsb")
            for g in range(4):
                tpv = tp_pool.tile([P, 512], d_av, tag="tp")
                for t4 in range(4):
                    j = 4 * g + t4
                    nc.tensor.transpose(tpv[:, t4 * C:(t4 + 1) * C],
                                        vt[:, j * P:(j + 1) * P],
                                        (ident_r if F32R_AV else ident)[0:C, 0:C])
                nc.vector.tensor_copy(vsb[:, 4 * g:4 * (g + 1), :], tpv[:, 0:4 * C])

            ogt = og_pool.tile([C, L], d_p, tag="og")
            og_tiles.append(ogt)

            # --- phase B: attention, software-pipelined at 2-q-tile chunks ---
            # Per chunk (2 q-tiles, 256 q): while the current chunk's S
            # matmuls/exps run, the PREVIOUS chunk's 16 transpose groups and
            # AV matmuls are interleaved between them so PE never idles on
            # the exp latency. AV consumes UNNORMALIZED E^T; out^T gets a
            # 1/rowsum broadcast fixup per chunk; attention tiles are
            # normalized in place afterwards, gating only their DMA store.
            NCH2 = NT // 2
            QW = 256  # chunk width in q

            def emit_y_tile(t):
                tsl = slice(t * P, (t + 1) * P)
                yp = tp_pool.tile([P, 512], f32, tag="tp")
                nc.tensor.matmul(yp[:, 0:O], lhsT=og_tiles[0][:, tsl],
                                 rhs=wo_sb[:, 0, :], start=True, stop=False)
                nc.tensor.matmul(yp[:, 0:O], lhsT=og_tiles[1][:, tsl],
                                 rhs=wo_sb[:, 1, :], start=False, stop=True)
                ysb = ypool.tile([P, O], f32, tag="ysb")
                nc.vector.tensor_add(ysb, yp[:, 0:O], b256)
                nc.sync.dma_start(out=y_d[tsl, :], in_=ysb)

            class Chunk:
                pass

            def emit_s_tile(cch, t2):
                t = 2 * cch + t2
                tsl = slice(t * P, (t + 1) * P)
                bias_sb = bias_pool.tile([P, L], d_bias, tag="bias")
                nc.sync.dma_start(out=bias_sb, in_=bias_d[hh, tsl, :])
                E = e_pool.tile([P, L], d_e, tag="e")
                acc = small.tile([P, 2], f32, tag="acc")
                qslice = qt[:, tsl]
                for half in range(2):
                    hs = slice(half * 1024, (half + 1) * 1024)
                    S = s_pool.tile([P, 1024], f32, tag="s")
                    for n in range(2):
                        ns = slice(half * 1024 + n * 512,
                                   half * 1024 + (n + 1) * 512)
                        nc.tensor.matmul(S[:, n * 512:(n + 1) * 512],
                                         lhsT=qslice, rhs=kt[:, ns],
                                         start=True, stop=False)
                    for n in range(2):
                        ns = slice(half * 1024 + n * 512,
                                   half * 1024 + (n + 1) * 512)
                        nc.tensor.matmul(S[:, n * 512:(n + 1) * 512],
                                         lhsT=ident_r, rhs=bias_sb[:, ns],
                                         start=False, stop=True)
                    nc.scalar.activation(E[:, hs], S, AF.Exp,
                                         accum_out=acc[:, half:half + 1])
                rsum = small.tile([P, 1], f32, tag="rsum")
                nc.vector.tensor_add(rsum, acc[:, 0:1], acc[:, 1:2])
                rec = small.tile([P, 1], f32, tag="rec")
                nc.vector.reciprocal(rec, rsum)
                return E, rec

            def start_chunk(cch):
                ck = Chunk()
                ck.cch = cch
                ck.e = []
                ck.rec = []
                ck.ot = ot_pool.tile([C, QW], f32, tag="ot")
                ck.at = []
                ck.next_av = 0
                return ck

            def emit_group(ck, j):
                # one k-block: 2 transposes -> copy -> (AV two groups back)
                tpa = tp_pool.tile([P, QW], d_e, tag="tp")
                for t2 in range(2):
                    nc.tensor.transpose(tpa[:, t2 * P:(t2 + 1) * P],
                                        ck.e[t2][:, j * P:(j + 1) * P],
                                        ident_r if F32R_TRANS else ident)
                aT = at_pool.tile([P, QW], d_av, tag="at")
                nc.vector.tensor_copy(aT, tpa)
                ck.at.append(aT)
                while ck.next_av <= len(ck.at) - 3:
                    jj = ck.next_av
                    nc.tensor.matmul(ck.ot, lhsT=vsb[:, jj, :], rhs=ck.at[jj],
                                     start=(jj == 0), stop=False)
                    ck.next_av += 1

            def emit_rts(ck):
                # 1/rowsum transposed to [1, QW] — emitted early so the
                # ones-matmul broadcast in finish_chunk never stalls PE.
                rtp = tp_pool.tile([P, QW], f32, tag="tp")
                for t2 in range(2):
                    nc.tensor.transpose(rtp[0:1, t2 * P:(t2 + 1) * P],
                                        ck.rec[t2], ident)
                ck.rts = small2.tile([1, QW], f32, tag="rts")
                nc.vector.tensor_copy(ck.rts, rtp[0:1, :])

            def finish_chunk(ck):
                for jj in range(ck.next_av, 16):
                    nc.tensor.matmul(ck.ot, lhsT=vsb[:, jj, :], rhs=ck.at[jj],
                                     start=(jj == 0), stop=(jj == 15))
                # R32[c, q-chunk] = broadcast of 1/rowsum over 32 partitions,
                # on GpSimd so PE does no work here.
                r32 = small2.tile([C, QW], f32, tag="r32")
                nc.gpsimd.partition_broadcast(r32, ck.rts)
                csl = slice(ck.cch * QW, (ck.cch + 1) * QW)
                nc.vector.tensor_mul(r32, ck.ot, r32)
                nc.vector.tensor_mul(ogt[:, csl], r32, sig[:, csl])
                for t2 in range(2):
                    t = 2 * ck.cch + t2
                    nc.vector.tensor_scalar_mul(ck.e[t2], ck.e[t2], ck.rec[t2])
                    nc.sync.dma_start(out=attn_d[hh, t * P:(t + 1) * P, :],
                                      in_=ck.e[t2])

            prev = None
            for cch in range(NCH2):
                cur = start_chunk(cch)
                for t2 in range(2):
                    E, rec = emit_s_tile(cch, t2)
                    cur.e.append(E)
                    cur.rec.append(rec)
                    if prev is not None:
                        if t2 == 1:
                            emit_rts(prev)
                        for j in range(8 * t2, 8 * t2 + 8):
                            emit_group(prev, j)
                if prev is not None:
                    finish_chunk(prev)
                    if hh == 1 and prev.cch >= 1:
                        emit_y_tile(2 * (prev.cch - 1))
                        emit_y_tile(2 * (prev.cch - 1) + 1)
                prev = cur
            # epilogue: drain the in-flight chunk + remaining y tiles
            for j in range(16):
                if j == 14:
                    emit_rts(prev)
                emit_group(prev, j)
            finish_chunk(prev)
            if hh == 1:
                for t in range(2 * (NCH2 - 2), NT):
                    emit_y_tile(t)

    nc.compile()
    return nc


def _get_nc():
    if "nc" not in _CACHE:
        _CACHE["nc"] = _build_program()
    return _CACHE["nc"]


def make_in_maps(q_inputs, kv_inputs, bias, qg_weights, kv_weights,
                 qg_bias, kv_bias, o_weights, o_bias):
    f = np.float32
    q_inputs = np.asarray(q_inputs, f)
    kv_inputs = np.asarray(kv_inputs, f)
    bias = np.asarray(bias, f)
    qg_weights = np.asarray(qg_weights, f)
    kv_weights = np.asarray(kv_weights, f)
    qg_bias = np.asarray(qg_bias, f)
    kv_bias = np.asarray(kv_bias, f)
    o_weights = np.asarray(o_weights, f)
    o_bias = np.asarray(o_bias, f)
    in_maps = []
    for c in range(NCORES):
        b, i = c // 4, c % 4
        hs = slice(2 * i, 2 * i + 2)
        in_maps.append({
            "xq": q_inputs[b],
            "xkv": kv_inputs[b],
            "biasH": bias[b, hs],
            "wqg": np.ascontiguousarray(np.moveaxis(qg_weights[:, 0, hs, :], 1, 0)),
            "wkv": np.ascontiguousarray(np.moveaxis(kv_weights[:, 0, hs, :], 1, 0)),
            "bqg": np.ascontiguousarray(qg_bias[0, hs, 0, :]),
            "bkv": np.ascontiguousarray(kv_bias[0, hs, 0, :]),
            "wo": np.ascontiguousarray(o_weights[0, hs]),
            "ob": (np.ascontiguousarray(o_bias[:, 0:1].T) if c in (0, 4)
                   else np.zeros((1, O), f)),
        })
    return in_maps


def assemble(results):
    y = np.zeros((B, L, O), np.float32)
    attn = np.empty((B, L, L, H), np.float32)
    for c in range(NCORES):
        b, i = c // 4, c % 4
        y[b] += results[c]["y_o"]
        a = results[c]["attn_o"]
        attn[b, :, :, 2 * i] = a[0]
        attn[b, :, :, 2 * i + 1] = a[1]
    return y, attn


def kernel(**inputs):
    from concourse.bass_utils import run_bass_kernel_spmd
    nc = _get_nc()
    in_maps = make_in_maps(**inputs)
    res = run_bass_kernel_spmd(nc, in_maps, list(range(NCORES)))
    return assemble(res.results)


def kernel_profiled(**inputs):
    """Like kernel(), but captures an NTFF profile. Returns (outputs, exec_ns, trace)."""
    from concourse.bass_utils import run_bass_kernel_spmd
    nc = _get_nc()
    in_maps = make_in_maps(**inputs)
    res = run_bass_kernel_spmd(nc, in_maps, list(range(NCORES)), trace=True)
    trace_path = None
    if res.instructions_and_trace is not None:
        trace_path = res.instructions_and_trace[1]
    return assemble(res.results), res.exec_time_ns, trace_path
